# revision 7
# baseline (speedup 1.0000x reference)
"""Trainium2 Bass kernel for a 2-head MultiHeadAttention + residual + LayerNorm.

Reference computation (per batch element b):
    h_i  = softmax((x Wq_i + bq_i)(x Wk_i + bk_i)^T / sqrt(DK)) (x Wv_i + bv_i)
    out  = LayerNorm(concat(h1, h2) Wo + bo + x) * gamma + beta

Sharding: pure data-parallel over batch. B=32 across 8 NeuronCores -> 4 batch
elements per core, weights replicated, no collectives.

Per-core pipeline (per batch element, layouts are [partition, free]):
    xT [h,s]  via PE transpose of x
    qT,kT [d,s] = Wq^T @ x^T      (fp32r matmuls, full PE rate at N>=256)
    v [s,d]     = x @ Wv
    scoresT [t,s] = k @ q^T; E^T = exp(scoresT/16) evicted as bf16 by ScalarE
    rowsum[s,1] via PE: lhsT=E^T tile (bf16 weights, FWL), rhs=ones -> 1/rowsum
    attT [d,s]  = (v^T-as-lhsT) @ E^T   (unnormalized)
    P_h [s,h]   = attT_h^T @ Wo_h  into separate PSUMs per head
    y = P1*r1 + P2*r2 + x  (fused scalar_tensor_tensor, per-partition scalars)
    out = gamma * (y-mu)*rstd + beta  (bn_stats/bn_aggr + fused ACT)
"""

import numpy as np

import concourse.bass as bass
from concourse import bacc
import concourse.mybir as mybir
import concourse.tile as tile
from concourse.bass_utils import run_bass_kernel_spmd
from concourse.masks import make_identity

F32 = mybir.dt.float32
F32R = mybir.dt.float32r
BF16 = mybir.dt.bfloat16
AF = mybir.ActivationFunctionType
ALU = mybir.AluOpType

N_CORES = 8
B, S, H, DK = 32, 1024, 512, 256
EPS = 1e-5
BL = B // N_CORES  # batch per core

P = 128          # partitions
NB = 512         # psum bank free size (fp32)
ST = S // P      # s-tiles (8)
SB = S // NB     # s-banks (2)
HT = H // P      # h-tiles (4)
DT = DK // P     # d-tiles per head (2)


# float32r: fp32 streamed at full PE rate (1 cyc/row for N>=256). The BIR
# verifier requires every matmul operand to be *produced* as float32r
# (rounded), so operand tiles are declared F32R and written by ACT/DVE ops.


def build_nc(bl=BL, s=S, h_dim=H, dk=DK, *, use_bq=True, use_bk=True,
             use_bv=True, use_bo=True, use_gamma=True, use_beta=True):
    """Build the per-core Bass program. Shapes parameterized for testing."""
    st, sb, ht, dt = s // P, s // NB, h_dim // P, dk // P
    d2t = 2 * dk // P
    scale = 1.0 / float(np.sqrt(dk))

    nc = bacc.Bacc("TRN2", target_bir_lowering=False, debug=False)

    x_d = nc.dram_tensor("inputs", [bl, s, h_dim], F32, kind="ExternalInput").ap()
    w_d, b_d = [], []
    for hh in (1, 2):
        for nm in ("q", "k", "v"):
            w_d.append(nc.dram_tensor(f"W{nm}{hh}", [h_dim, dk], F32,
                                      kind="ExternalInput").ap())
            b_d.append(nc.dram_tensor(f"b{nm}{hh}", [dk], F32,
                                      kind="ExternalInput").ap())
    wo_d = nc.dram_tensor("Wo", [2 * dk, h_dim], F32, kind="ExternalInput").ap()
    bo_d = nc.dram_tensor("bo", [h_dim], F32, kind="ExternalInput").ap()
    gamma_d = nc.dram_tensor("gamma", [h_dim], F32, kind="ExternalInput").ap()
    beta_d = nc.dram_tensor("beta", [h_dim], F32, kind="ExternalInput").ap()
    out_d = nc.dram_tensor("out", [bl, s, h_dim], F32, kind="ExternalOutput").ap()

    def bcast(ap, n):
        """[n]-DRAM vector broadcast to [P, n] (partition step 0)."""
        return bass.AP(tensor=ap.tensor, offset=ap.offset,
                       ap=[[0, P]] + [list(p) for p in ap.ap])

    with tile.TileContext(nc) as tc:
        with (
            tc.tile_pool(name="const", bufs=1) as const,
            tc.tile_pool(name="px", bufs=2) as px,
            tc.tile_pool(name="pxT", bufs=1) as pxT,
            tc.tile_pool(name="pqk", bufs=1) as pqk,
            tc.tile_pool(name="pv", bufs=2) as pv,
            tc.tile_pool(name="pE", bufs=2) as pE,
            tc.tile_pool(name="patt", bufs=4) as patt,
            tc.tile_pool(name="pr", bufs=4) as pr,
            tc.tile_pool(name="py", bufs=2) as py,
            tc.tile_pool(name="pout", bufs=4) as pout,
            tc.tile_pool(name="pps", bufs=3, space="PSUM") as pps,
            tc.tile_pool(name="ppr", bufs=2, space="PSUM") as ppr,
            tc.tile_pool(name="ppp", bufs=3, space="PSUM") as ppp,
        ):
            # ---- constants ----
            ident = const.tile([P, P], F32, tag="ident")
            make_identity(nc, ident)
            ones_bf = const.tile([P, 1], BF16, tag="ones")
            nc.vector.memset(ones_bf, 1.0)
            eps_t = const.tile([P, 1], F32, tag="eps")
            nc.vector.memset(eps_t, EPS)

            wqkv = []   # per head: (wq, wk, wv) tiles [P, ht, dk] f32r
            bqkv = []   # per head: (bq, bk) tiles [P, dt] or None
            bv_b = []   # per head: [P, dk] broadcast or None
            use_b = {"q": use_bq, "k": use_bk, "v": use_bv}
            for hh in range(2):
                ws, bs, vb = [], [], None
                for j, nm in enumerate(("q", "k", "v")):
                    wd = w_d[hh * 3 + j]
                    wtmp = px.tile([P, ht, dk], F32, tag="wtmp")
                    nc.gpsimd.dma_start(
                        out=wtmp, in_=wd.rearrange("(kt p) d -> p kt d", p=P))
                    w_t = const.tile([P, ht, dk], F32R, tag=f"w{nm}{hh}")
                    nc.vector.tensor_copy(out=w_t, in_=wtmp)
                    ws.append(w_t)
                    bd = b_d[hh * 3 + j]
                    if nm in ("q", "k") and use_b[nm]:
                        b_t = const.tile([P, dt], F32, tag=f"b{nm}{hh}")
                        nc.sync.dma_start(
                            out=b_t, in_=bd.rearrange("(m p) -> p m", p=P))
                        bs.append(b_t)
                    elif nm in ("q", "k"):
                        bs.append(None)
                    elif use_b[nm]:  # v bias: broadcast along partitions
                        vb = const.tile([P, dk], F32, tag=f"bv{hh}")
                        nc.sync.dma_start(out=vb, in_=bcast(bd, dk))
                wqkv.append(ws)
                bqkv.append(bs)
                bv_b.append(vb)

            wo_f = const.tile([P, d2t, h_dim], F32, tag="wo_f")
            nc.sync.dma_start(
                out=wo_f, in_=wo_d.rearrange("(kt p) h -> p kt h", p=P))
            wo_b = const.tile([P, d2t, h_dim], BF16, tag="wo_b")
            nc.vector.tensor_copy(out=wo_b, in_=wo_f)

            bo_t = None
            if use_bo:
                bo_t = const.tile([P, h_dim], F32, tag="bo_b")
                nc.sync.dma_start(out=bo_t, in_=bcast(bo_d, h_dim))
            gamma_t = None
            if use_gamma:
                gamma_t = const.tile([P, h_dim], F32, tag="gamma_b")
                nc.sync.dma_start(out=gamma_t, in_=bcast(gamma_d, h_dim))
            beta_t = None
            if use_beta:
                beta_t = const.tile([P, h_dim], F32, tag="beta_b")
                nc.sync.dma_start(out=beta_t, in_=bcast(beta_d, h_dim))

            # ---- main loop over local batch ----
            for b in range(bl):
                # x natural [s-part, h]: one DMA
                xn = px.tile([P, st, h_dim], F32, tag="xn")
                nc.sync.dma_start(
                    out=xn, in_=x_d[b].rearrange("(t p) h -> p t h", p=P))

                # x^T [h-part, s] via PE transpose, 4 blocks per psum bank
                xT = pxT.tile([P, ht, s], F32R, tag="xT")
                for hh_t in range(ht):
                    for g in range(st // 4):
                        ps = pps.tile([P, NB], F32, tag="ps")
                        for j in range(4):
                            t_i = g * 4 + j
                            nc.tensor.transpose(
                                ps[:, j * P:(j + 1) * P],
                                xn[:, t_i, hh_t * P:(hh_t + 1) * P],
                                ident)
                        nc.scalar.activation(
                            out=xT[:, hh_t, g * NB:(g + 1) * NB], in_=ps,
                            func=AF.Copy)

                attT = []   # per head [P, dt, s] bf16
                rrec = []   # per head [P, st] f32 (1/rowsum per s-tile)
                for hh in range(2):
                    wq_t, wk_t, wv_t = wqkv[hh]
                    bq_t, bk_t = bqkv[hh]

                    # qT, kT [d-part, s] = W^T @ x^T
                    qT = pqk.tile([P, dt, s], F32R, tag="qT")
                    kT = pqk.tile([P, dt, s], F32R, tag="kT")
                    for dst, w_t, b_t in ((qT, wq_t, bq_t), (kT, wk_t, bk_t)):
                        for m in range(dt):
                            for sbi in range(sb):
                                ps = pps.tile([P, NB], F32, tag="ps")
                                for kt in range(ht):
                                    nc.tensor.matmul(
                                        ps,
                                        lhsT=(w_t[:, kt, m * P:(m + 1) * P]),
                                        rhs=(xT[:, kt, sbi * NB:(sbi + 1) * NB]),
                                        start=(kt == 0), stop=(kt == ht - 1))
                                o = dst[:, m, sbi * NB:(sbi + 1) * NB]
                                if b_t is not None:
                                    nc.scalar.activation(
                                        out=o, in_=ps, func=AF.Identity,
                                        bias=b_t[:, m:m + 1])
                                else:
                                    nc.scalar.activation(out=o, in_=ps,
                                                         func=AF.Copy)

                    # v [s-part, d] = x @ Wv  (evicted as bf16)
                    vt = pv.tile([P, st, dk], BF16, tag="vt")
                    for t_i in range(st):
                        psv = pps.tile([P, NB], F32, tag="ps")
                        for kt in range(ht):
                            nc.tensor.matmul(
                                psv[:, 0:dk],
                                lhsT=(xT[:, kt, t_i * P:(t_i + 1) * P]),
                                rhs=(wv_t[:, kt, :]),
                                start=(kt == 0), stop=(kt == ht - 1))
                        if bv_b[hh] is not None:
                            nc.vector.tensor_tensor(
                                out=vt[:, t_i, :], in0=psv[:, 0:dk],
                                in1=bv_b[hh], op=ALU.add)
                        else:
                            nc.vector.tensor_copy(out=vt[:, t_i, :],
                                                  in_=psv[:, 0:dk])

                    # scoresT [t-part, s] = k @ q^T ; E^T = exp(scale*scoresT)
                    ET = pE.tile([P, st, s], BF16, tag="ET")
                    for t_i in range(st):
                        for sbi in range(sb):
                            ps = pps.tile([P, NB], F32, tag="ps")
                            for kt in range(dt):
                                nc.tensor.matmul(
                                    ps,
                                    lhsT=(kT[:, kt, t_i * P:(t_i + 1) * P]),
                                    rhs=(qT[:, kt, sbi * NB:(sbi + 1) * NB]),
                                    start=(kt == 0), stop=(kt == dt - 1))
                            nc.scalar.activation(
                                out=ET[:, t_i, sbi * NB:(sbi + 1) * NB],
                                in_=ps, func=AF.Exp, scale=scale)

                    # 1/rowsum [s-part, 1] via PE: lhsT = E^T tile, rhs = ones
                    rr = pr.tile([P, st], F32, tag=f"rrec{hh}")
                    for s_i in range(st):
                        psr = ppr.tile([P, 1], F32, tag="psr")
                        for t_i in range(st):
                            nc.tensor.matmul(
                                psr,
                                lhsT=ET[:, t_i, s_i * P:(s_i + 1) * P],
                                rhs=ones_bf,
                                start=(t_i == 0), stop=(t_i == st - 1))
                        nc.vector.reciprocal(out=rr[:, s_i:s_i + 1], in_=psr)
                    rrec.append(rr)

                    # attT [d-part, s] = v^T @ E (unnormalized)
                    at = patt.tile([P, dt, s], BF16, tag=f"attT{hh}")
                    for m in range(dt):
                        for sbi in range(sb):
                            ps = pps.tile([P, NB], F32, tag="ps")
                            for t_i in range(st):
                                nc.tensor.matmul(
                                    ps,
                                    lhsT=vt[:, t_i, m * P:(m + 1) * P],
                                    rhs=ET[:, t_i, sbi * NB:(sbi + 1) * NB],
                                    start=(t_i == 0), stop=(t_i == st - 1))
                            nc.scalar.activation(
                                out=at[:, m, sbi * NB:(sbi + 1) * NB],
                                in_=ps, func=AF.Copy)
                    attT.append(at)

                # output projection + combine + residual + layernorm, per s-tile
                for t_i in range(st):
                    pps_h = []
                    for hh in range(2):
                        pp = ppp.tile([P, h_dim], F32, tag="pp")
                        for kt in range(dt):
                            nc.tensor.matmul(
                                pp,
                                lhsT=attT[hh][:, kt, t_i * P:(t_i + 1) * P],
                                rhs=wo_b[:, hh * dt + kt, :],
                                start=(kt == 0), stop=(kt == dt - 1))
                        pps_h.append(pp)

                    t1 = py.tile([P, h_dim], F32, tag="t1")
                    nc.vector.scalar_tensor_tensor(
                        out=t1, in0=pps_h[1], scalar=rrec[1][:, t_i:t_i + 1],
                        in1=xn[:, t_i, :], op0=ALU.mult, op1=ALU.add)
                    y = py.tile([P, h_dim], F32, tag="y")
                    nc.vector.scalar_tensor_tensor(
                        out=y, in0=pps_h[0], scalar=rrec[0][:, t_i:t_i + 1],
                        in1=t1, op0=ALU.mult, op1=ALU.add)
                    if bo_t is not None:
                        nc.vector.tensor_tensor(out=y, in0=y, in1=bo_t,
                                                op=ALU.add)

                    stats = py.tile([P, 6], F32, tag="stats")
                    nc.vector.bn_stats(stats, y)
                    mv = py.tile([P, 2], F32, tag="mv")
                    nc.vector.bn_aggr(mv, stats)
                    sd = py.tile([P, 1], F32, tag="sd")
                    nc.scalar.activation(out=sd, in_=mv[:, 1:2], func=AF.Sqrt,
                                         bias=eps_t)
                    rstd = py.tile([P, 1], F32, tag="rstd")
                    nc.vector.reciprocal(rstd, sd)
                    # -mu * rstd
                    nmr = py.tile([P, 1], F32, tag="nmr")
                    nc.vector.tensor_scalar(
                        out=nmr, in0=mv[:, 0:1], scalar1=rstd, scalar2=-1.0,
                        op0=ALU.mult, op1=ALU.mult)

                    ot = pout.tile([P, h_dim], F32, tag="ot")
                    if gamma_t is None and beta_t is None:
                        nc.scalar.activation(out=ot, in_=y, func=AF.Identity,
                                             bias=nmr, scale=rstd)
                    else:
                        t2 = py.tile([P, h_dim], F32, tag="t2")
                        nc.scalar.activation(out=t2, in_=y, func=AF.Identity,
                                             bias=nmr, scale=rstd)
                        if gamma_t is not None and beta_t is not None:
                            nc.vector.tensor_tensor(out=t2, in0=t2,
                                                    in1=gamma_t, op=ALU.mult)
                            nc.vector.tensor_tensor(out=ot, in0=t2,
                                                    in1=beta_t, op=ALU.add)
                        elif gamma_t is not None:
                            nc.vector.tensor_tensor(out=ot, in0=t2,
                                                    in1=gamma_t, op=ALU.mult)
                        else:
                            nc.vector.tensor_tensor(out=ot, in0=t2,
                                                    in1=beta_t, op=ALU.add)
                    nc.sync.dma_start(
                        out=out_d[b, t_i * P:(t_i + 1) * P, :], in_=ot)

    nc.compile()
    return nc


def _run(arrs, **spmd_kwargs):
    x = arrs["inputs"]
    assert x.shape == (B, S, H), x.shape

    nc = build_nc(
        use_bq=bool(np.any(arrs["bq1"]) or np.any(arrs["bq2"])),
        use_bk=bool(np.any(arrs["bk1"]) or np.any(arrs["bk2"])),
        use_bv=bool(np.any(arrs["bv1"]) or np.any(arrs["bv2"])),
        use_bo=bool(np.any(arrs["bo"])),
        use_gamma=bool(np.any(arrs["gamma"] != 1.0)),
        use_beta=bool(np.any(arrs["beta"])),
    )

    shared = {k: v for k, v in arrs.items() if k != "inputs"}
    in_maps = [dict(shared, inputs=x[i * BL:(i + 1) * BL])
               for i in range(N_CORES)]
    res = run_bass_kernel_spmd(nc, in_maps, core_ids=list(range(N_CORES)),
                               **spmd_kwargs)
    out = np.concatenate([res.results[i]["out"] for i in range(N_CORES)],
                         axis=0)
    return out.astype(np.float32), res


def kernel(**inputs):
    arrs = {k: np.ascontiguousarray(np.asarray(v, dtype=np.float32))
            for k, v in inputs.items()}
    out, _ = _run(arrs)
    return out
